# revision 37
# baseline (speedup 1.0000x reference)
"""Trainium2 Bass kernel for nn_ButterflyFactorNewMlp.

Computes: attn = einsum('ds,td->st', w1, w2) * sparse_mask
          out  = gelu(einsum('bds,st->bdt', x, attn) + b2)   (exact erf gelu)

Key structural facts (hardcoded):
  - x: [64, 768, 729] f32; w1: [2916, 729]; w2: [729, 2916]; b2: [729]
  - sparse_mask is block-diagonal: mask[s,t] != 0  iff  s//81 == t//81 and
    (s%27)//3 == (t%27)//3.  So attn = blockdiag of 9 diagonal 81x81 blocks.
  - Sharding: data-parallel on batch (8 batches per core).  The small attn
    computation is replicated on every core (fp16 weights, ~8.6MB DMA) —
    measured, this beats d-sharding + AllReduce: any on-device collective
    drags in a ~20us ncfw startup + ~45us kernel-entry barrier (launch-skew
    sync) + ~18us latency-bound AllReduce, serializing ~100us before the
    attn-dependent matmuls can start.

Precision: x and the weights are rounded to fp16 (rel err ~2.4e-4 per
element, on par with the PE's fp32r mode); contractions accumulate in fp32
PSUM, gelu runs on the fp32 accumulator via the ScalarE erf-gelu LUT (LUT
error ~2e-6), and the output store rounds to fp16.  End-to-end absmax error
~7e-4 relative to scale.

Contraction layout (s' space, padded to 768 wide): the host inserts a ones
column at position 405 and appends another at 730, so the bias rows ride
the contraction at s'=405 (serving output cols 0:384) and s'=730 (cols
384:729).  Features s >= 405 shift by one.

  stage 1 (replicated attn): for each 128-row s'-chunk j of attn, the only
  t-columns its rows touch span < 244 columns (block-diagonal mask).  attn
  chunk j accumulates 23 [128d x cw]^T @ [128d x 244] fp16 matmuls; the
  kd-outer loop keeps all six chunks' PSUM banks in flight so the matmuls
  pipeline with the weight DMA stream.  A DVE multiply by a precomputed
  mask window writes the masked result straight into the chunk's rows
  (partition-aligned); b2 is cast-DMA'd into the two bias rows.

  stage 2: x^T tiles [s'-chunk, 1024 tokens] are produced directly by the
  hardware DMA-transpose engine (fp16 makes the xbar path legal — this is
  why there are no PE transposes), and with attn block-diagonal each output
  half needs just 4 matmuls per 128-token tile:
    half A (t 0:384):   s'-chunks 0,1,2 full (K=128) + chunk3 rows 0:22
    half B (t 383:729): chunk2 rows 64:128 + chunks 3,4 full + chunk5 0:91
  (half B is 346 wide from 383: column 383 is computed twice and copied out
  once, keeping both halves' free dims >= 256.)  Exact-gelu runs straight
  out of PSUM and stores are fp16.
"""

import sys

if "/opt/trn_rl_repo" not in sys.path:
    sys.path.insert(0, "/opt/trn_rl_repo")

import numpy as np

import concourse.bacc as bacc
import concourse.bass as bass
import concourse.mybir as mybir
import concourse.tile as tile
from concourse.bass import ds, ts
from concourse.bass_utils import run_bass_kernel_spmd
from concourse.masks import make_identity

F32 = mybir.dt.float32
F16 = mybir.dt.float16
GELU = mybir.ActivationFunctionType.Gelu

N_CORES = 8
B, D, S = 64, 768, 729          # batch, channels, features (729 = in = out)
H = 2916                        # hidden dim of the weight contraction
SP = 736                        # s' width: 729 features + 2 ones + pad
M_PER_CORE = (B // N_CORES) * D  # 6144 tokens per core
M_TILE = 128
M_BATCH = 4                      # m-tiles per x DMA / output store
N_ITERS = M_PER_CORE // (M_TILE * M_BATCH)
HP = 2944                        # hidden padded to 23*128
N_KD = HP // 128                 # 23 contraction chunks for the attn matmuls
KD_BATCH = 2                     # kd chunks per weight DMA (~375KB)
T_SIZES = [128, 128, 128, 128, 128, 92]  # valid rows per s'-chunk
# stage-1 window starts: chunk j's masked columns live in [WC0[j], WC0[j]+244)
WIN = 244
WC0 = [0, 81, 243, 324, 485, 485]
# main-matmul plan: (chunk j, partition base, K) per output half.
# The t-split at 405 is exactly the block-5 boundary, so half A covers
# blocks 0-4 (s' < 406 plus the biasA row) and half B covers blocks 5-8
# (chunk 3's rows 0:21 carry only block-4/biasA values, which are zero for
# t >= 405, so chunk 3 can be taken in full) — 7 matmuls per token tile.
MM_A = [(0, 0, 128), (1, 0, 128), (2, 0, 128), (3, 0, 22)]
MM_B = [(3, 0, 128), (4, 0, 128), (5, 0, 91)]
T0 = 405
T1 = 324

_COMPILED = None
LAST = None  # BassKernelResults of the most recent kernel() call (for test.py)


def _build():
    nc = bacc.Bacc("TRN2", target_bir_lowering=False, debug=False)

    x_d = nc.dram_tensor("x", [M_PER_CORE, SP], F16, kind="ExternalInput")
    w1_d = nc.dram_tensor("w1s", [HP, SP], F16, kind="ExternalInput")
    w2t_d = nc.dram_tensor("w2ts", [HP, S], F16, kind="ExternalInput")
    mw_d = nc.dram_tensor("maskw", [6, 128, WIN], F16, kind="ExternalInput")
    b2_d = nc.dram_tensor("b2", [1, S], F32, kind="ExternalInput")
    out_d = nc.dram_tensor("out", [M_PER_CORE, S], F16, kind="ExternalOutput")

    with tile.TileContext(nc) as tc:
        with (
            tc.tile_pool(name="const", bufs=1) as cpool,
            tc.tile_pool(name="xin", bufs=6) as xpool,
            tc.tile_pool(name="xt", bufs=24) as xtpool,
            tc.tile_pool(name="oout", bufs=3) as opool,
            tc.tile_pool(name="tpsum", bufs=2, space="PSUM") as tpsum,
            tc.tile_pool(name="mpsum", bufs=6, space="PSUM") as mpsum,
        ):
            # ---------------- stage 1: replicated attn ----------------
            w1_sb = cpool.tile([128, N_KD, SP], F16)
            w2_sb = cpool.tile([128, N_KD, S], F16)
            xsb_pre = {}
            for kb in range((N_KD + KD_BATCH - 1) // KD_BATCH):
                k0 = kb * KD_BATCH
                kn = min(KD_BATCH, N_KD - k0)
                nc.sync.dma_start(
                    w1_sb[:, ds(k0, kn), :],
                    w1_d[ds(k0 * 128, kn * 128), :].rearrange(
                        "(c p) f -> p c f", p=128
                    ),
                )
                nc.sync.dma_start(
                    w2_sb[:, ds(k0, kn), :],
                    w2t_d[ds(k0 * 128, kn * 128), :].rearrange(
                        "(c p) f -> p c f", p=128
                    ),
                )
                # interleave the first x tiles with the weight stream so the
                # PE has transpose work while stage 1 waits on weights
                if kb % 4 == 3 and kb // 4 < 3:
                    it = kb // 4
                    xsb_pre[it] = xpool.tile(
                        [128, M_BATCH, SP], F16, tag="x_sb", name=f"xpre{it}"
                    )
                    nc.sync.dma_start(
                        xsb_pre[it][:],
                        x_d[
                            ds(it * M_TILE * M_BATCH, M_TILE * M_BATCH), :
                        ].rearrange("(c p) f -> p c f", p=128),
                    )
            mw_sb = cpool.tile([128, 6, WIN], F16)
            nc.sync.dma_start(mw_sb[:], mw_d[:].rearrange("c p f -> p c f"))

            # attn in SBUF (fp16), s' coordinates; chunk width padded to 730
            # so the f32-bitcast memset is exact
            attn_all = cpool.tile([128, 6, S + 1], F16)
            nc.gpsimd.memset(attn_all[:].bitcast(F32), 0.0)

            # kd-outer so each chunk group's accumulations pipeline with
            # the incoming weight DMA stream (one PSUM bank per chunk);
            # weights stay SBUF-resident so the second pass re-reads SBUF
            psws = {}
            for j in range(6):
                psws[j] = mpsum.tile([128, WIN], F32, tag="ps", name=f"psw{j}")
            for kd in range(N_KD):
                for j in range(6):
                    nc.tensor.matmul(
                        psws[j][0 : T_SIZES[j], :],
                        w1_sb[:, kd, ds(128 * j, T_SIZES[j])],
                        w2_sb[:, kd, ds(WC0[j], WIN)],
                        start=(kd == 0),
                        stop=(kd == N_KD - 1),
                    )
            for j in range(6):
                cw = T_SIZES[j]
                nc.vector.tensor_tensor(
                    attn_all[0:cw, j, ds(WC0[j], WIN)],
                    psws[j][0:cw, :],
                    mw_sb[0:cw, j, :],
                    mybir.AluOpType.mult,
                )
            # bias rows (cast f32 -> f16 during DMA, SWDGE)
            nc.gpsimd.dma_start(attn_all[21:22, 3, 0:T0], b2_d[:, 0:T0])
            nc.gpsimd.dma_start(attn_all[90:91, 5, T0:S], b2_d[:, T0:S])

            # ---------------- stage 2: main matmul ----------------
            identf = cpool.tile([128, 128], F32)
            make_identity(nc, identf[:])
            ident = cpool.tile([128, 128], F16)
            nc.vector.tensor_copy(ident[:], identf[:])

            for it in range(N_ITERS):
                if it in xsb_pre:
                    x_sb = xsb_pre[it]
                else:
                    x_sb = xpool.tile([128, M_BATCH, SP], F16)
                    nc.sync.dma_start(
                        x_sb[:],
                        x_d[
                            ds(it * M_TILE * M_BATCH, M_TILE * M_BATCH), :
                        ].rearrange("(c p) f -> p c f", p=128),
                    )
                o_sb = opool.tile([128, M_BATCH, S], F16, tag="o")
                for sub in range(M_BATCH):
                    # all 6 transposes land in a single PSUM bank (fp16)
                    tp = tpsum.tile([128, 768], F16, tag="tp", name="tp")
                    for j in range(6):
                        ksz = T_SIZES[j]
                        nc.tensor.transpose(
                            tp[0:ksz, ts(j, 128)],
                            x_sb[:, sub, ds(128 * j, ksz)],
                            ident[:],
                        )
                    xt = xtpool.tile([128, 768], F16, tag="xt")
                    nc.vector.tensor_copy(xt[:], tp[:])

                    psA = mpsum.tile([128, T0], F32, tag="ps", name="psA")
                    psB = mpsum.tile([128, T1], F32, tag="ps", name="psB")
                    for ps_t, t0, tw, plan in [
                        (psA, 0, T0, MM_A),
                        (psB, T0, T1, MM_B),
                    ]:
                        for n, (j, p0, ksz) in enumerate(plan):
                            nc.tensor.matmul(
                                ps_t[:],
                                xt[p0 : p0 + ksz, ts(j, 128)],
                                attn_all[p0 : p0 + ksz, j, ds(t0, tw)],
                                start=(n == 0),
                                stop=(n == len(plan) - 1),
                            )

                    nc.scalar.activation(o_sb[:, sub, 0:T0], psA[:], GELU)
                    nc.scalar.activation(o_sb[:, sub, T0:S], psB[:], GELU)
                nc.sync.dma_start(
                    out_d[ds(it * M_TILE * M_BATCH, M_TILE * M_BATCH), :].rearrange(
                        "(c p) f -> p c f", p=128
                    ),
                    o_sb[:],
                )

    nc.compile()
    return nc


def _host_prep(w1, w2, mask):
    """Build the s'-layout fp16 weight/mask-window tables (layout only)."""
    w1s = np.zeros((HP, SP), np.float16)
    w1s[:H, 0:405] = w1[:, 0:405]
    w1s[:H, 406:730] = w1[:, 405:729]
    w2ts = np.zeros((HP, S), np.float16)
    w2ts[:H] = w2.T
    maskw = np.zeros((6, 128, WIN), np.float16)
    for j in range(6):
        for p in range(128):
            sp = 128 * j + p
            if sp < 405:
                s = sp
            elif 406 <= sp <= 729:
                s = sp - 1
            else:
                continue  # bias/pad rows
            maskw[j, p, :] = mask[s, WC0[j] : WC0[j] + WIN]
    return w1s, w2ts, maskw


def kernel(x, w1, w2, b2, sparse_mask):
    global _COMPILED, LAST
    if _COMPILED is None:
        _COMPILED = _build()
    nc = _COMPILED

    x = np.asarray(x, dtype=np.float32)
    w1 = np.asarray(w1, dtype=np.float32)
    w2 = np.asarray(w2, dtype=np.float32)
    b2 = np.asarray(b2, dtype=np.float32)
    mask = np.asarray(sparse_mask, dtype=np.float32)

    w1s, w2ts, maskw = _host_prep(w1, w2, mask)
    b2r = np.ascontiguousarray(b2.reshape(1, S))

    xf = x.reshape(B * D, S)
    xt = np.zeros((B * D, SP), np.float16)
    xt[:, 0:405] = xf[:, 0:405]
    xt[:, 405] = 1.0
    xt[:, 406:730] = xf[:, 405:729]
    xt[:, 730] = 1.0
    in_maps = []
    for c in range(N_CORES):
        in_maps.append(
            {
                "x": xt[c * M_PER_CORE : (c + 1) * M_PER_CORE],
                "w1s": w1s,
                "w2ts": w2ts,
                "maskw": maskw,
                "b2": b2r,
            }
        )

    LAST = run_bass_kernel_spmd(nc, in_maps, list(range(N_CORES)))
    out = np.concatenate([LAST.results[c]["out"] for c in range(N_CORES)], axis=0)
    return out.reshape(B, D, S).astype(np.float32)


# revision 38
# speedup vs baseline: 1.0890x; 1.0890x over previous
"""Trainium2 Bass kernel for nn_ButterflyFactorNewMlp.

Computes: attn = einsum('ds,td->st', w1, w2) * sparse_mask
          out  = gelu(einsum('bds,st->bdt', x, attn) + b2)   (exact erf gelu)

Key structural facts (hardcoded):
  - x: [64, 768, 729] f32; w1: [2916, 729]; w2: [729, 2916]; b2: [729]
  - sparse_mask is block-diagonal: mask[s,t] != 0  iff  s//81 == t//81 and
    (s%27)//3 == (t%27)//3.  So attn = blockdiag of 9 diagonal 81x81 blocks.
  - Sharding: data-parallel on batch (8 batches per core).  The small attn
    computation is replicated on every core (fp16 weights, ~8.6MB DMA) —
    measured, this beats d-sharding + AllReduce: any on-device collective
    drags in a ~20us ncfw startup + ~45us kernel-entry barrier (launch-skew
    sync) + ~18us latency-bound AllReduce, serializing ~100us before the
    attn-dependent matmuls can start.

Precision: x and the weights are rounded to fp16 (rel err ~2.4e-4 per
element, on par with the PE's fp32r mode); contractions accumulate in fp32
PSUM, gelu runs on the fp32 accumulator via the ScalarE erf-gelu LUT (LUT
error ~2e-6), and the output store rounds to fp16.  End-to-end absmax error
~7e-4 relative to scale.

Contraction layout (s' space, padded to 768 wide): the host inserts a ones
column at position 405 and appends another at 730, so the bias rows ride
the contraction at s'=405 (serving output cols 0:384) and s'=730 (cols
384:729).  Features s >= 405 shift by one.

  stage 1 (replicated attn): for each 128-row s'-chunk j of attn, the only
  t-columns its rows touch span < 244 columns (block-diagonal mask).  attn
  chunk j accumulates 23 [128d x cw]^T @ [128d x 244] fp16 matmuls; the
  kd-outer loop keeps all six chunks' PSUM banks in flight so the matmuls
  pipeline with the weight DMA stream.  A DVE multiply by a precomputed
  mask window writes the masked result straight into the chunk's rows
  (partition-aligned); b2 is cast-DMA'd into the two bias rows.

  stage 2: x^T tiles [s'-chunk, 1024 tokens] are produced directly by the
  hardware DMA-transpose engine (fp16 makes the xbar path legal — this is
  why there are no PE transposes), and with attn block-diagonal each output
  half needs just 4 matmuls per 128-token tile:
    half A (t 0:384):   s'-chunks 0,1,2 full (K=128) + chunk3 rows 0:22
    half B (t 383:729): chunk2 rows 64:128 + chunks 3,4 full + chunk5 0:91
  (half B is 346 wide from 383: column 383 is computed twice and copied out
  once, keeping both halves' free dims >= 256.)  Exact-gelu runs straight
  out of PSUM and stores are fp16.
"""

import sys

if "/opt/trn_rl_repo" not in sys.path:
    sys.path.insert(0, "/opt/trn_rl_repo")

import numpy as np

import concourse.bacc as bacc
import concourse.bass as bass
import concourse.mybir as mybir
import concourse.tile as tile
from concourse.bass import ds, ts
from concourse.bass_utils import run_bass_kernel_spmd
from concourse.masks import make_identity

F32 = mybir.dt.float32
F16 = mybir.dt.float16
GELU = mybir.ActivationFunctionType.Gelu

N_CORES = 8
B, D, S = 64, 768, 729          # batch, channels, features (729 = in = out)
H = 2916                        # hidden dim of the weight contraction
SP = 736                        # s' width: 729 features + 2 ones + pad
M_PER_CORE = (B // N_CORES) * D  # 6144 tokens per core
M_TILE = 128
M_BATCH = 4                      # m-tiles per x DMA / output store
N_ITERS = M_PER_CORE // (M_TILE * M_BATCH)
HP = 2944                        # hidden padded to 23*128
N_KD = HP // 128                 # 23 contraction chunks for the attn matmuls
KD_BATCH = 4                     # kd chunks per weight DMA (~750KB)
T_SIZES = [128, 128, 128, 128, 128, 92]  # valid rows per s'-chunk
# stage-1 window starts: chunk j's masked columns live in [WC0[j], WC0[j]+244)
WIN = 244
WC0 = [0, 81, 243, 324, 485, 485]
# main-matmul plan: (chunk j, partition base, K) per output half.
# The t-split at 405 is exactly the block-5 boundary, so half A covers
# blocks 0-4 (s' < 406 plus the biasA row) and half B covers blocks 5-8
# (chunk 3's rows 0:21 carry only block-4/biasA values, which are zero for
# t >= 405, so chunk 3 can be taken in full) — 7 matmuls per token tile.
MM_A = [(0, 0, 128), (1, 0, 128), (2, 0, 128), (3, 0, 22)]
MM_B = [(3, 0, 128), (4, 0, 128), (5, 0, 91)]
T0 = 405
T1 = 324

_COMPILED = None
LAST = None  # BassKernelResults of the most recent kernel() call (for test.py)


def _build():
    nc = bacc.Bacc("TRN2", target_bir_lowering=False, debug=False)

    x_d = nc.dram_tensor("x", [M_PER_CORE, SP], F16, kind="ExternalInput")
    w1_d = nc.dram_tensor("w1s", [HP, SP], F16, kind="ExternalInput")
    w2t_d = nc.dram_tensor("w2ts", [HP, S], F16, kind="ExternalInput")
    mw_d = nc.dram_tensor("maskw", [6, 128, WIN], F16, kind="ExternalInput")
    b2_d = nc.dram_tensor("b2", [1, S], F32, kind="ExternalInput")
    out_d = nc.dram_tensor("out", [M_PER_CORE, S], F16, kind="ExternalOutput")

    with tile.TileContext(nc) as tc:
        with (
            tc.tile_pool(name="const", bufs=1) as cpool,
            tc.tile_pool(name="xin", bufs=6) as xpool,
            tc.tile_pool(name="xt", bufs=24) as xtpool,
            tc.tile_pool(name="oout", bufs=3) as opool,
            tc.tile_pool(name="tpsum", bufs=4, space="PSUM") as tpsum,
            tc.tile_pool(name="mpsum", bufs=4, space="PSUM") as mpsum,
        ):
            # ---------------- stage 1: replicated attn ----------------
            w1_sb = cpool.tile([128, N_KD, SP], F16)
            w2_sb = cpool.tile([128, N_KD, S], F16)
            for kb in range((N_KD + KD_BATCH - 1) // KD_BATCH):
                k0 = kb * KD_BATCH
                kn = min(KD_BATCH, N_KD - k0)
                nc.sync.dma_start(
                    w1_sb[:, ds(k0, kn), :],
                    w1_d[ds(k0 * 128, kn * 128), :].rearrange(
                        "(c p) f -> p c f", p=128
                    ),
                )
                nc.sync.dma_start(
                    w2_sb[:, ds(k0, kn), :],
                    w2t_d[ds(k0 * 128, kn * 128), :].rearrange(
                        "(c p) f -> p c f", p=128
                    ),
                )
            mw_sb = cpool.tile([128, 6, WIN], F16)
            nc.sync.dma_start(mw_sb[:], mw_d[:].rearrange("c p f -> p c f"))

            # attn in SBUF (fp16), s' coordinates; chunk width padded to 730
            # so the f32-bitcast memset is exact
            attn_all = cpool.tile([128, 6, S + 1], F16)
            nc.gpsimd.memset(attn_all[:].bitcast(F32), 0.0)

            # kd-outer so each chunk group's accumulations pipeline with
            # the incoming weight DMA stream (one PSUM bank per chunk);
            # weights stay SBUF-resident so the second pass re-reads SBUF
            psws = {}
            for grp in ([0, 1, 2, 3], [4, 5]):
                for j in grp:
                    psws[j] = mpsum.tile(
                        [128, WIN], F32, tag="ps", name=f"psw{j}"
                    )
                for kd in range(N_KD):
                    for j in grp:
                        nc.tensor.matmul(
                            psws[j][0 : T_SIZES[j], :],
                            w1_sb[:, kd, ds(128 * j, T_SIZES[j])],
                            w2_sb[:, kd, ds(WC0[j], WIN)],
                            start=(kd == 0),
                            stop=(kd == N_KD - 1),
                        )
            for j in range(6):
                cw = T_SIZES[j]
                nc.vector.tensor_tensor(
                    attn_all[0:cw, j, ds(WC0[j], WIN)],
                    psws[j][0:cw, :],
                    mw_sb[0:cw, j, :],
                    mybir.AluOpType.mult,
                )
            # bias rows (cast f32 -> f16 during DMA, SWDGE)
            nc.gpsimd.dma_start(attn_all[21:22, 3, 0:T0], b2_d[:, 0:T0])
            nc.gpsimd.dma_start(attn_all[90:91, 5, T0:S], b2_d[:, T0:S])

            # ---------------- stage 2: main matmul ----------------
            identf = cpool.tile([128, 128], F32)
            make_identity(nc, identf[:])
            ident = cpool.tile([128, 128], F16)
            nc.vector.tensor_copy(ident[:], identf[:])

            for it in range(N_ITERS):
                x_sb = xpool.tile([128, M_BATCH, SP], F16)
                nc.sync.dma_start(
                    x_sb[:],
                    x_d[ds(it * M_TILE * M_BATCH, M_TILE * M_BATCH), :].rearrange(
                        "(c p) f -> p c f", p=128
                    ),
                )
                o_sb = opool.tile([128, M_BATCH, S], F16, tag="o")
                for sub in range(M_BATCH):
                    # all 6 transposes land in a single PSUM bank (fp16)
                    tp = tpsum.tile([128, 768], F16, tag="tp", name="tp")
                    for j in range(6):
                        ksz = T_SIZES[j]
                        nc.tensor.transpose(
                            tp[0:ksz, ts(j, 128)],
                            x_sb[:, sub, ds(128 * j, ksz)],
                            ident[:],
                        )
                    xt = xtpool.tile([128, 768], F16, tag="xt")
                    nc.vector.tensor_copy(xt[:], tp[:])

                    psA = mpsum.tile([128, T0], F32, tag="ps", name="psA")
                    psB = mpsum.tile([128, T1], F32, tag="ps", name="psB")
                    for ps_t, t0, tw, plan in [
                        (psA, 0, T0, MM_A),
                        (psB, T0, T1, MM_B),
                    ]:
                        for n, (j, p0, ksz) in enumerate(plan):
                            nc.tensor.matmul(
                                ps_t[:],
                                xt[p0 : p0 + ksz, ts(j, 128)],
                                attn_all[p0 : p0 + ksz, j, ds(t0, tw)],
                                start=(n == 0),
                                stop=(n == len(plan) - 1),
                            )

                    nc.scalar.activation(o_sb[:, sub, 0:T0], psA[:], GELU)
                    nc.scalar.activation(o_sb[:, sub, T0:S], psB[:], GELU)
                nc.sync.dma_start(
                    out_d[ds(it * M_TILE * M_BATCH, M_TILE * M_BATCH), :].rearrange(
                        "(c p) f -> p c f", p=128
                    ),
                    o_sb[:],
                )

    nc.compile()
    return nc


def _host_prep(w1, w2, mask):
    """Build the s'-layout fp16 weight/mask-window tables (layout only)."""
    w1s = np.zeros((HP, SP), np.float16)
    w1s[:H, 0:405] = w1[:, 0:405]
    w1s[:H, 406:730] = w1[:, 405:729]
    w2ts = np.zeros((HP, S), np.float16)
    w2ts[:H] = w2.T
    maskw = np.zeros((6, 128, WIN), np.float16)
    for j in range(6):
        for p in range(128):
            sp = 128 * j + p
            if sp < 405:
                s = sp
            elif 406 <= sp <= 729:
                s = sp - 1
            else:
                continue  # bias/pad rows
            maskw[j, p, :] = mask[s, WC0[j] : WC0[j] + WIN]
    return w1s, w2ts, maskw


def kernel(x, w1, w2, b2, sparse_mask):
    global _COMPILED, LAST
    if _COMPILED is None:
        _COMPILED = _build()
    nc = _COMPILED

    x = np.asarray(x, dtype=np.float32)
    w1 = np.asarray(w1, dtype=np.float32)
    w2 = np.asarray(w2, dtype=np.float32)
    b2 = np.asarray(b2, dtype=np.float32)
    mask = np.asarray(sparse_mask, dtype=np.float32)

    w1s, w2ts, maskw = _host_prep(w1, w2, mask)
    b2r = np.ascontiguousarray(b2.reshape(1, S))

    xf = x.reshape(B * D, S)
    xt = np.zeros((B * D, SP), np.float16)
    xt[:, 0:405] = xf[:, 0:405]
    xt[:, 405] = 1.0
    xt[:, 406:730] = xf[:, 405:729]
    xt[:, 730] = 1.0
    in_maps = []
    for c in range(N_CORES):
        in_maps.append(
            {
                "x": xt[c * M_PER_CORE : (c + 1) * M_PER_CORE],
                "w1s": w1s,
                "w2ts": w2ts,
                "maskw": maskw,
                "b2": b2r,
            }
        )

    LAST = run_bass_kernel_spmd(nc, in_maps, list(range(N_CORES)))
    out = np.concatenate([LAST.results[c]["out"] for c in range(N_CORES)], axis=0)
    return out.reshape(B, D, S).astype(np.float32)
